# revision 36
# baseline (speedup 1.0000x reference)
"""Trainium2 Bass kernel for nn_Attention_28905129902499.

Dense transformer attention block (q/k/v proj + RoPE + causal GQA attention
+ o_proj), B=1, S=2048, HIDDEN=2048, 32 q heads / 8 kv heads, head_dim 64.

Sharding: tensor-parallel over heads across 8 NeuronCores. Core c owns
q heads 4c..4c+3 and kv head c. Each core computes its partial
out_c = attn_c @ wo[:, c*256:(c+1)*256].T  (shape [S, H]); the host sums the
8 partials (the tensor-parallel all-reduce) and returns the full output.

Device-side schedule (per core) — four pipelined phases:
  A0: qkv projection + RoPE for sequence half 0 (all of q/k/v), with the
      three 128-row output groups interleaved per contraction tile so the
      PE consumes x tiles at the rate the DMA delivers them.
  B:  attention + o_proj for chunks q0,q1 (they only need seq half 0 of
      k/v) — the ACT engine starts exp'ing ~20us into the kernel.
  A1: qkv + RoPE for half 1.
  D:  attention + o_proj for chunks q2,q3.
PSUM pools are scoped per phase (8-bank budget); SBUF x/w pools span A0-A1.

Other notes:
  - All device inputs are pre-converted to bf16 and pre-tiled on the host
    (dtype conversion + RoPE trig tables are host-side marshaling).
  - q/k are produced *transposed*: qT/kT [d, s] with head_dim on partitions,
    so attention scores are computed directly transposed.
  - softmax runs without max subtraction and the denominators come for free
    out of the PV matmul (V extended with 64 all-ones columns).
  - the causal triangle mask is a [128,128] gpsimd affine_select on just
    the diagonal block; fully-masked columns are skipped everywhere.
"""

import sys
import types
from contextlib import ExitStack

import numpy as np
import ml_dtypes

for _p in ("/opt/trn_rl_repo", "/root/.axon_site/_ro/trn_rl_repo"):
    if _p not in sys.path:
        sys.path.append(_p)

import concourse.bass as bass
import concourse.tile as tile
import concourse.mybir as mybir
from concourse.bass_utils import run_bass_kernel_spmd

dt = mybir.dt
AF = mybir.ActivationFunctionType
ALU = mybir.AluOpType
bf16 = ml_dtypes.bfloat16

# ---------------------------------------------------------------- constants
S = 2048          # sequence length
H = 2048          # hidden size
NH = 32           # query heads
NKV = 8           # kv heads
D = 64            # head dim
G = NH // NKV     # 4 query heads per kv head
N_CORES = 8
DQ = G * D        # 256 local q dims per core
MQKV = DQ + 2 * D   # 384 fused qkv output dims per core
KT = H // 128     # 16 contraction tiles
NS = S // 512     # 4 sequence chunks of 512
KB = S // 128     # 16 key blocks of 128
SCALE = 1.0 / np.sqrt(D)
ROPE_BASE = 10000.0


def _split_multi_waits(nc):
    """The walrus build in this container accepts only ONE sync-wait per
    instruction; Tile emits more. Move extras onto same-engine NOPs placed
    immediately before the instruction (same-engine streams are in-order, so
    this is semantically identical)."""
    for bb in nc.main_func.blocks:
        insts = bb.instructions
        i = 0
        while i < len(insts):
            ins = insts[i]
            si = ins.sync_info
            waits = list(si.on_wait) if si is not None else []
            if len(waits) > 1:
                for w in waits[:-1]:
                    nop = mybir.InstNoOp(
                        name=nc.get_next_instruction_name(),
                        engine=ins.engine,
                        bass_nofuse=True,
                        sync_info=mybir.SyncInfo(on_wait=[w], on_update=[]),
                    )
                    nc.register_instruction(nop, overwrite=True)
                    insts.insert(i, nop)
                    i += 1
                ins.sync_info = mybir.SyncInfo(
                    on_wait=[waits[-1]], on_update=list(si.on_update)
                )
            i += 1


def _install_profile_hook():
    """Register the NTFF profile hook the agent image's antenv lacks, so
    run_bass_kernel_spmd(trace=True) can return HW exec times."""
    try:
        import antenv.axon_hooks  # noqa: F401
        return
    except ImportError:
        pass
    hook = None
    try:
        from trn_agent_boot.trn_boot import _ntff_profile_via_ctypes
        hook = _ntff_profile_via_ctypes("/opt/axon/libaxon_pjrt.so")
    except Exception:
        hook = None
    m = types.ModuleType("antenv.axon_hooks")
    m.get_axon_ntff_profile_hook = lambda: hook
    m.set_axon_ntff_profile_hook = lambda h: None
    sys.modules["antenv.axon_hooks"] = m


def hlo_sl(h):
    return slice(64 * (h % 2), 64 * (h % 2) + 64)


# ---------------------------------------------------------------- program
def build_program():
    nc = bass.Bass()

    # all inputs host-pre-tiled AND host-pre-converted to bf16
    xT = nc.declare_dram_parameter("xT", [128, KT * S], dt.bfloat16, isOutput=False)
    wqkvT = nc.declare_dram_parameter("wqkvT", [128, KT * MQKV], dt.bfloat16, isOutput=False)
    woT = nc.declare_dram_parameter("woT", [128, 2 * S], dt.bfloat16, isOutput=False)
    cosT = nc.declare_dram_parameter("cosT", [128, S], dt.bfloat16, isOutput=False)
    sinT = nc.declare_dram_parameter("sinT", [128, S], dt.bfloat16, isOutput=False)
    rt2 = nc.declare_dram_parameter("rt2", [128, 128], dt.bfloat16, isOutput=False)
    poutT = nc.declare_dram_parameter("poutT", [H, S], dt.bfloat16, isOutput=True)

    with tile.TileContext(nc) as tc, ExitStack() as stack:
        # ---------------- persistent pools / consts ----------------
        const_pool = stack.enter_context(tc.tile_pool(name="const", bufs=1))
        rt_b = const_pool.tile([128, 128], dt.bfloat16, tag="rtb")
        nc.gpsimd.dma_start(rt_b[:], rt2[:])

        trig_pool = stack.enter_context(tc.tile_pool(name="trig", bufs=1))
        cos_rep = trig_pool.tile([128, S], dt.bfloat16, tag="cosr")
        sin_rep = trig_pool.tile([128, S], dt.bfloat16, tag="sinr")
        nc.gpsimd.dma_start(cos_rep[:], cosT[:])
        nc.gpsimd.dma_start(sin_rep[:], sinT[:])

        wo_pool = stack.enter_context(tc.tile_pool(name="wop", bufs=1))
        wo_b = [wo_pool.tile([128, S], dt.bfloat16, tag=f"wo{k}", name=f"wo{k}")
                for k in range(2)]
        for k in range(2):
            nc.gpsimd.dma_start(wo_b[k][:], woT[:, S * k:S * (k + 1)])

        # attention operand tiles
        att_pool = stack.enter_context(tc.tile_pool(name="att", bufs=1))
        qrope = [att_pool.tile([128, S], dt.bfloat16, tag=f"qrope{p}", name=f"qrope{p}")
                 for p in range(2)]
        kropeE = att_pool.tile([128, S], dt.bfloat16, tag="kropeE")
        kropeO = att_pool.tile([128, S], dt.bfloat16, tag="kropeO")
        nc.gpsimd.memset(kropeE[64:128, :], 0.0)
        nc.gpsimd.memset(kropeO[0:64, :], 0.0)
        vextA = att_pool.tile([128, S], dt.bfloat16, tag="vextA")
        vextB = att_pool.tile([128, S], dt.bfloat16, tag="vextB")
        nc.gpsimd.memset(vextA[:], 1.0)
        nc.gpsimd.memset(vextB[:], 1.0)
        vT_sb = att_pool.tile([128, S], dt.bfloat16, tag="vTsb")
        attnT = [att_pool.tile([128, S], dt.bfloat16, tag=f"attnT{p}", name=f"attnT{p}")
                 for p in range(2)]
        vA3 = vextA.rearrange("p (kb j) -> p kb j", kb=KB)

        # attention-phase SBUF scratch (persistent across phases)
        esb = stack.enter_context(tc.tile_pool(name="exp_sb", bufs=4))
        nsb = stack.enter_context(tc.tile_pool(name="norm_sb", bufs=2))
        osb = stack.enter_context(tc.tile_pool(name="out_sb", bufs=2))
        pout3 = poutT.rearrange("(mm p) j -> p mm j", p=128)

        # ---------------- x / wqkv loads (SBUF pool spans A0..A1) ----------
        xw_scope = ExitStack()
        proj_pool = xw_scope.enter_context(tc.tile_pool(name="proj", bufs=1))
        wqkv_big = proj_pool.tile([128, KT * MQKV], dt.bfloat16, tag="wqkvb")
        xt_pool = xw_scope.enter_context(tc.tile_pool(name="xtb", bufs=1))
        # x as [128, 1024] tiles: (k, half); half 0 loads first on both queues
        xt2 = [[xt_pool.tile([128, 1024], dt.bfloat16, tag=f"xt{k}h{hf}",
                             name=f"xt{k}h{hf}") for hf in range(2)]
               for k in range(KT)]
        rsc = xw_scope.enter_context(tc.tile_pool(name="rope_sc", bufs=2))

        def q_eng(i):
            return nc.sync if i % 2 == 0 else nc.scalar

        # wqkv chunk ch covers k in [2ch, 2ch+2); interleave with x so the
        # weights arrive just ahead of the x tiles that need them
        for ch in range(2):
            q_eng(ch).dma_start(
                wqkv_big[:, 2 * ch * MQKV:2 * (ch + 1) * MQKV],
                wqkvT[:, 2 * ch * MQKV:2 * (ch + 1) * MQKV])
        for k in range(KT):
            if k in (1, 3, 5):
                for ch in (k + 1, k + 2):
                    q_eng(ch).dma_start(
                        wqkv_big[:, 2 * ch * MQKV:2 * (ch + 1) * MQKV],
                        wqkvT[:, 2 * ch * MQKV:2 * (ch + 1) * MQKV])
            q_eng(k).dma_start(
                xt2[k][0][:], xT[:, k * S:k * S + 1024])

        def load_x_half1():
            # issued after the half-0 rope so these 16 descriptor issues
            # don't occupy the scalar engine ahead of the rope casts; the
            # sync queue has ~40us of slack before half 1 is consumed
            for k in range(KT):
                nc.sync.dma_start(
                    xt2[k][1][:], xT[:, k * S + 1024:k * S + 2048])

        def wqkv_sl(k, m):
            return wqkv_big[:, k * MQKV + 128 * m:k * MQKV + 128 * (m + 1)]

        # ---------------- phase A: qkv + RoPE for one sequence half --------
        def qkv_half(half, qpsum, rpsum):
            # six [128,512] psum groups, one per (m, n2); n2=0 groups stop
            # halfway through the phase so the ACT casts (and the whole
            # RoPE chain) start while the n2=1 matmuls still stream
            pss = {}
            for m in (2, 0, 1):
                for n2 in range(2):
                    pss[(m, n2)] = qpsum.tile(
                        [128, 512], dt.float32, tag=f"qkvps{m}n{n2}",
                        name=f"qkvps{m}n{n2}")
            # k/v and the first q pair stream first (k-interleaved, paced to
            # DMA arrival); m=1 runs after, so its matmuls overlap the
            # DVE/ACT rope work on the m=2/m=0 outputs
            for n2 in range(2):
                for k in range(KT):
                    for m in (2, 0):
                        nc.tensor.matmul(
                            pss[(m, n2)][:],
                            wqkv_sl(k, m),
                            xt2[k][half][:, 512 * n2:512 * (n2 + 1)],
                            start=(k == 0), stop=(k == KT - 1),
                        )
            for n2 in range(2):
                for k in range(KT):
                    nc.tensor.matmul(
                        pss[(1, n2)][:],
                        wqkv_sl(k, 1),
                        xt2[k][half][:, 512 * n2:512 * (n2 + 1)],
                        start=(k == 0), stop=(k == KT - 1),
                    )
            # n2=0 chunks first: attention chunk q0/q1 depends on them
            for m, n2 in ((2, 0), (0, 0), (1, 0), (2, 1), (0, 1), (1, 1)):
                nrows = 128 if m < 2 else 64
                ps = pss[(m, n2)]
                n = 2 * half + n2
                sl = slice(512 * n, 512 * (n + 1))
                if m == 2:
                    nc.scalar.copy(vT_sb[64:128, sl], ps[64:128, :])
                qc = rsc.tile([128, 512], dt.float32, tag="qc", name="qc")
                nc.vector.tensor_tensor(out=qc[:nrows, :], in0=ps[:nrows, :],
                                        in1=cos_rep[:nrows, sl], op=ALU.mult)
                # bf16 cast for the PE rotate matmul runs on ACT
                qraw = rsc.tile([128, 512], dt.bfloat16, tag="qraw", name="qraw")
                nc.scalar.copy(qraw[:nrows, :], ps[:nrows, :])
                rot = rpsum.tile([128, 512], dt.float32, tag="rot", name="rot")
                nc.tensor.matmul(rot[:nrows, :], rt_b[:nrows, :nrows],
                                 qraw[:nrows, :], start=True, stop=True)
                qs = rsc.tile([128, 512], dt.float32, tag="qs", name="qs")
                nc.vector.tensor_tensor(out=qs[:nrows, :], in0=rot[:nrows, :],
                                        in1=sin_rep[:nrows, sl], op=ALU.mult)
                dst = qrope[m] if m < 2 else kropeE
                nc.vector.tensor_tensor(out=dst[:nrows, sl], in0=qc[:nrows, :],
                                        in1=qs[:nrows, :], op=ALU.add)
            # k duplicate + v transposes for this half
            hsl = slice(1024 * half, 1024 * (half + 1))
            nc.gpsimd.dma_start(kropeO[64:128, hsl], kropeE[0:64, hsl])
            nc.sync.dma_start_transpose(
                vA3[:, 8 * half:8 * (half + 1), 0:64], vT_sb[64:128, hsl])
            for kb in range(8 * half, 8 * half + 8):
                nc.gpsimd.dma_start(
                    vextB[:, 128 * kb + 64:128 * (kb + 1)],
                    vextA[:, 128 * kb:128 * kb + 64])

        # -------- phase B/D: attention + o_proj for one 512-col chunk ------
        def attn_chunk(q, spsum, vpsum, filler=None):
            """filler: list of closures (o_proj m-tile units for an earlier,
            already-normalized chunk); one is emitted after each kb2 pair so
            the in-order PE always has independent matmuls to run while this
            chunk's PV waits on ACT's exp."""
            filler = list(filler) if filler else []
            n_fill_total = len(filler)
            n_units_total = 4 * (4 * q + 4) // 2
            unit_idx = [0]
            fill_done = [0]
            qsl = slice(512 * q, 512 * (q + 1))
            nkb = 4 * q + 4          # kb blocks this chunk attends to
            pv_even = None
            for h in range(4):
                pair = h // 2
                par = h % 2      # 0: even head (pv rows 0-63), 1: odd
                krope = kropeE if par == 0 else kropeO
                vext = vextA if par == 0 else vextB
                pv = vpsum.tile([128, 512], dt.float32, tag="pv", name="pv")

                def emit_pv(kb2, ex, los):
                    for j in (0, 1):
                        kb = kb2 + j
                        lo = los[j]
                        nc.tensor.matmul(
                            pv[:, lo:512],
                            vext[:, 128 * kb:128 * (kb + 1)],
                            ex[:, 512 * j + lo:512 * (j + 1)],
                            start=(kb == 0), stop=(kb == nkb - 1),
                            skip_group_check=True)

                # software pipeline: scores/exp for pair i+1 are emitted
                # before the PV of pair i, so the in-order PE never waits
                # on ACT's exp of the pair it just scored
                pending = None
                for kb2 in range(0, nkb, 2):
                    sc = spsum.tile([128, 1024], dt.float32, tag="scps",
                                    name="scps")
                    ex = esb.tile([128, 1024], dt.bfloat16, tag="expp",
                                  name="expp")
                    los = []
                    for j in (0, 1):
                        kb = kb2 + j
                        lo = 128 * (kb - 4 * q) if kb >= 4 * q else 0
                        los.append(lo)
                        nc.tensor.matmul(
                            sc[:, 512 * j + lo:512 * (j + 1)],
                            krope[:, 128 * kb:128 * (kb + 1)],
                            qrope[pair][:, 512 * q + lo:512 * (q + 1)],
                            start=True, stop=True)
                    if los == [0, 0]:
                        nc.scalar.activation(ex[:], sc[:], AF.Exp,
                                             scale=float(SCALE))
                    else:
                        for j in (0, 1):
                            nc.scalar.activation(
                                ex[:, 512 * j + los[j]:512 * (j + 1)],
                                sc[:, 512 * j + los[j]:512 * (j + 1)],
                                AF.Exp, scale=float(SCALE))
                    for j in (0, 1):
                        kb = kb2 + j
                        if kb >= 4 * q:
                            # triangular causal mask on the [128,128]
                            # diagonal block: keep iff col >= partition
                            lo = los[j]
                            nc.gpsimd.affine_select(
                                out=ex[:, 512 * j + lo:512 * j + lo + 128],
                                in_=ex[:, 512 * j + lo:512 * j + lo + 128],
                                compare_op=ALU.is_ge, fill=0.0,
                                base=0, pattern=[[1, 128]],
                                channel_multiplier=-1)
                    # Bresenham-spread the filler units across the chunk,
                    # between this pair's scores and the previous pair's PV:
                    # the PE chews the filler while ACT exps the new scores
                    unit_idx[0] += 1
                    want = unit_idx[0] * n_fill_total // n_units_total
                    while fill_done[0] < want and filler:
                        filler.pop(0)()
                        fill_done[0] += 1
                    if pending is not None:
                        emit_pv(*pending)
                    pending = (kb2, ex, los)
                if pending is not None:
                    emit_pv(*pending)
                if par == 0:
                    pv_even = pv
                    continue
                # paired normalize for heads (h-1, h): one Ln + one Exp on
                # a full [128,512] tile instead of two per-head passes.
                # pv_even: pv rows 0-63, sums 64-127; pv (odd): flipped.
                pvmix = nsb.tile([128, 512], dt.float32, tag="pvmix", name="pvmix")
                nc.vector.tensor_copy(pvmix[0:64, :], pv_even[0:64, :])
                nc.vector.tensor_copy(pvmix[64:128, :], pv[64:128, :])
                summix = nsb.tile([128, 512], dt.float32, tag="summix", name="summix")
                nc.vector.tensor_copy(summix[0:64, :], pv[0:64, :])
                nc.vector.tensor_copy(summix[64:128, :], pv_even[64:128, :])
                lns = nsb.tile([128, 512], dt.float32, tag="lns", name="lns")
                nc.scalar.activation(lns[:], summix[:], AF.Ln)
                # partition-swap the halves so each rcp row aligns with its pv
                lnd = nsb.tile([128, 512], dt.float32, tag="lnd", name="lnd")
                # sync queue, not gpsimd: keep gpsimd free for the affines
                nc.sync.dma_start(lnd[0:64, :], lns[64:128, :])
                nc.sync.dma_start(lnd[64:128, :], lns[0:64, :])
                rcp = nsb.tile([128, 512], dt.float32, tag="rcp", name="rcp")
                nc.scalar.activation(rcp[:], lnd[:], AF.Exp, scale=-1.0)
                nc.vector.tensor_tensor(
                    out=attnT[pair][:, qsl],
                    in0=pvmix[:], in1=rcp[:], op=ALU.mult)
            while filler:
                filler.pop(0)()

        def oproj_units(q, opsum, copy_eng="mix"):
            """16 closures, one per o_proj m-tile of chunk q (2 matmuls +
            psum->bf16 copy; every 8th also fires the staged out-DMA).
            copy_eng='dve' keeps all copies off ACT — use for filler units
            that run inside an exp-saturated attention chunk."""
            qsl = slice(512 * q, 512 * (q + 1))
            state = {}

            def make_unit(m):
                mh, mm = m // 8, m % 8

                def unit():
                    if mm == 0:
                        state[mh] = osb.tile([128, 8 * 512], dt.bfloat16,
                                             tag="ob", name="ob")
                    ob = state[mh]
                    ps = opsum.tile([128, 512], dt.float32, tag="ops", name="ops")
                    for kd in range(2):
                        nc.tensor.matmul(
                            ps[:],
                            wo_b[kd][:, 128 * m:128 * (m + 1)],
                            attnT[kd][:, qsl],
                            start=(kd == 0), stop=(kd == 1))
                    if copy_eng == "dve" or mm % 2 == 0:
                        nc.vector.tensor_copy(ob[:, 512 * mm:512 * (mm + 1)], ps[:])
                    else:
                        nc.scalar.copy(ob[:, 512 * mm:512 * (mm + 1)], ps[:])
                    if mm == 7:
                        ob3 = ob.rearrange("p (mm j) -> p mm j", mm=8)
                        if q == NS - 1 and mh == 1:
                            # last transfer: split across both queues
                            nc.sync.dma_start(pout3[:, 8:12, qsl], ob3[:, 0:4, :])
                            nc.scalar.dma_start(pout3[:, 12:16, qsl], ob3[:, 4:8, :])
                        else:
                            eng = nc.sync if (q + mh) % 2 == 0 else nc.scalar
                            eng.dma_start(pout3[:, 8 * mh:8 * (mh + 1), qsl],
                                          ob3[:, :, :])
                return unit

            return [make_unit(m) for m in range(KT)]

        def oproj_chunk(q, opsum):
            for u in oproj_units(q, opsum):
                u()

        # ---------------- the pipeline ----------------
        with tc.tile_pool(name="qkv_psum0", bufs=1, space="PSUM") as qpsum0, \
             tc.tile_pool(name="rot_psum0", bufs=2, space="PSUM") as rpsum0:
            qkv_half(0, qpsum0, rpsum0)
            load_x_half1()
        with tc.tile_pool(name="sc_psum0", bufs=2, space="PSUM") as spsum0, \
             tc.tile_pool(name="pv_psum0", bufs=2, space="PSUM") as vpsum0, \
             tc.tile_pool(name="op_psum0", bufs=2, space="PSUM") as opsum0:
            attn_chunk(0, spsum0, vpsum0)
            attn_chunk(1, spsum0, vpsum0, filler=oproj_units(0, opsum0))
            # o_proj q1 is deferred into phase D as attn q2's filler, so
            # phase B's psum drains fast and A1 isn't blocked on bank WARs
        with tc.tile_pool(name="qkv_psum1", bufs=1, space="PSUM") as qpsum1, \
             tc.tile_pool(name="rot_psum1", bufs=2, space="PSUM") as rpsum1:
            qkv_half(1, qpsum1, rpsum1)
        xw_scope.close()
        with tc.tile_pool(name="sc_psum1", bufs=2, space="PSUM") as spsum1, \
             tc.tile_pool(name="pv_psum1", bufs=2, space="PSUM") as vpsum1, \
             tc.tile_pool(name="op_psum1", bufs=2, space="PSUM") as opsum1:
            attn_chunk(2, spsum1, vpsum1,
                       filler=oproj_units(1, opsum1, copy_eng="dve"))
            attn_chunk(3, spsum1, vpsum1,
                       filler=oproj_units(2, opsum1, copy_eng="dve"))
            oproj_chunk(3, opsum1)

    _split_multi_waits(nc)
    return nc


_PROGRAM = None


def _get_program():
    global _PROGRAM
    if _PROGRAM is None:
        _PROGRAM = build_program()
    return _PROGRAM


# ---------------------------------------------------------------- host side
def make_inputs(hidden_states, position_ids, wq, wk, wv, wo):
    """Shard + marshal full inputs into per-core DRAM parameter maps.

    All dtype conversion (fp32 -> bf16) and the RoPE trig tables are done
    here on the host; the device kernel only sees bf16 operands."""
    x = np.asarray(hidden_states, dtype=np.float32).reshape(S, H)
    # pre-tiled [128, KT*S]: row p, col k*S+j  =  xT[k*128+p, j] = x[j, k*128+p]
    xT = np.ascontiguousarray(
        x.T.reshape(KT, 128, S).transpose(1, 0, 2).reshape(128, KT * S)
    ).astype(bf16)

    # RoPE trig tables [128, S]: partition p covers q/k dim (p % 64) of a
    # head; inv_freq index is (p % 64) % 32 == p % 32
    pos = np.asarray(position_ids).reshape(S).astype(np.float64)
    inv_freq = 1.0 / (ROPE_BASE ** (np.arange(0, D, 2, dtype=np.float64) / D))
    ang = pos[None, :] * inv_freq[np.arange(128) % 32][:, None]  # [128, S]
    cosT = np.cos(ang).astype(bf16)
    sinT = np.sin(ang).astype(bf16)

    # rotation matrix RT2 [128, 128]: block-diag pair of RT [64, 64] where
    # (RT.T @ v)[j] = -v[j+32] for j<32, v[j-32] for j>=32  (rotate_half)
    R = np.zeros((D, D), dtype=np.float32)
    for j in range(32):
        R[j + 32, j] = -1.0       # out[j] = -in[j+32]
        R[j, j + 32] = 1.0        # out[j+32] = in[j]
    RT2 = np.zeros((128, 128), dtype=np.float32)
    RT2[0:64, 0:64] = R
    RT2[64:128, 64:128] = R
    RT2 = RT2.astype(bf16)

    wq = np.asarray(wq, dtype=np.float32)
    wk = np.asarray(wk, dtype=np.float32)
    wv = np.asarray(wv, dtype=np.float32)
    wo = np.asarray(wo, dtype=np.float32)

    in_maps = []
    for c in range(N_CORES):
        wq_c = wq[DQ * c:DQ * (c + 1)]           # [256, H]
        wk_c = wk[D * c:D * (c + 1)]             # [64, H]
        wv_c = wv[D * c:D * (c + 1)]             # [64, H]
        wqkvT_c = np.concatenate([wq_c, wk_c, wv_c], axis=0).T   # [H, 384]
        wqkvT_c = np.ascontiguousarray(
            wqkvT_c.reshape(KT, 128, MQKV).transpose(1, 0, 2)
            .reshape(128, KT * MQKV)).astype(bf16)
        woT_c = wo[:, DQ * c:DQ * (c + 1)].T                 # [256, H]
        woT_c = np.ascontiguousarray(
            woT_c.reshape(2, 128, H).transpose(1, 0, 2).reshape(128, 2 * H)
        ).astype(bf16)
        in_maps.append({
            "xT": xT,
            "wqkvT": wqkvT_c,
            "woT": woT_c,
            "cosT": cosT,
            "sinT": sinT,
            "rt2": RT2,
        })
    return in_maps


def kernel(hidden_states, position_ids, wq, wk, wv, wo):
    _install_profile_hook()
    nc = _get_program()
    in_maps = make_inputs(hidden_states, position_ids, wq, wk, wv, wo)
    res = run_bass_kernel_spmd(nc, in_maps, list(range(N_CORES)))
    acc = np.zeros((H, S), dtype=np.float32)
    for c in range(N_CORES):
        acc += res.results[c]["poutT"].astype(np.float32)
    return np.ascontiguousarray(acc.T)[None, :, :]


if __name__ == "__main__":
    rng = np.random.default_rng(0)
    hs = rng.standard_normal((1, S, H), dtype=np.float32)
    pid = np.broadcast_to(np.arange(S, dtype=np.int64)[None, :], (1, S))
    std = 1.0 / np.sqrt(H)
    w_q = (rng.standard_normal((NH * D, H), dtype=np.float32) * std)
    w_k = (rng.standard_normal((NKV * D, H), dtype=np.float32) * std)
    w_v = (rng.standard_normal((NKV * D, H), dtype=np.float32) * std)
    w_o = (rng.standard_normal((H, NH * D), dtype=np.float32) * std)
    out = kernel(hs, pid, w_q, w_k, w_v, w_o)
    print("out", out.shape, out.dtype, float(np.abs(out).mean()))


# revision 39
# speedup vs baseline: 1.0225x; 1.0225x over previous
"""Trainium2 Bass kernel for nn_Attention_28905129902499.

Dense transformer attention block (q/k/v proj + RoPE + causal GQA attention
+ o_proj), B=1, S=2048, HIDDEN=2048, 32 q heads / 8 kv heads, head_dim 64.

Sharding: tensor-parallel over heads across 8 NeuronCores. Core c owns
q heads 4c..4c+3 and kv head c. Each core computes its partial
out_c = attn_c @ wo[:, c*256:(c+1)*256].T  (shape [S, H]); the host sums the
8 partials (the tensor-parallel all-reduce) and returns the full output.

Device-side schedule (per core) — four pipelined phases:
  A0: qkv projection + RoPE for sequence half 0 (all of q/k/v), with the
      three 128-row output groups interleaved per contraction tile so the
      PE consumes x tiles at the rate the DMA delivers them.
  B:  attention + o_proj for chunks q0,q1 (they only need seq half 0 of
      k/v) — the ACT engine starts exp'ing ~20us into the kernel.
  A1: qkv + RoPE for half 1.
  D:  attention + o_proj for chunks q2,q3.
PSUM pools are scoped per phase (8-bank budget); SBUF x/w pools span A0-A1.

Other notes:
  - All device inputs are pre-converted to bf16 and pre-tiled on the host
    (dtype conversion + RoPE trig tables are host-side marshaling).
  - q/k are produced *transposed*: qT/kT [d, s] with head_dim on partitions,
    so attention scores are computed directly transposed.
  - softmax runs without max subtraction and the denominators come for free
    out of the PV matmul (V extended with 64 all-ones columns).
  - the causal triangle mask is a [128,128] gpsimd affine_select on just
    the diagonal block; fully-masked columns are skipped everywhere.
"""

import sys
import types
from contextlib import ExitStack

import numpy as np
import ml_dtypes

for _p in ("/opt/trn_rl_repo", "/root/.axon_site/_ro/trn_rl_repo"):
    if _p not in sys.path:
        sys.path.append(_p)

import concourse.bass as bass
import concourse.tile as tile
import concourse.mybir as mybir
from concourse.bass_utils import run_bass_kernel_spmd

dt = mybir.dt
AF = mybir.ActivationFunctionType
ALU = mybir.AluOpType
bf16 = ml_dtypes.bfloat16

# ---------------------------------------------------------------- constants
S = 2048          # sequence length
H = 2048          # hidden size
NH = 32           # query heads
NKV = 8           # kv heads
D = 64            # head dim
G = NH // NKV     # 4 query heads per kv head
N_CORES = 8
DQ = G * D        # 256 local q dims per core
MQKV = DQ + 2 * D   # 384 fused qkv output dims per core
KT = H // 128     # 16 contraction tiles
NS = S // 512     # 4 sequence chunks of 512
KB = S // 128     # 16 key blocks of 128
SCALE = 1.0 / np.sqrt(D)
ROPE_BASE = 10000.0


def _split_multi_waits(nc):
    """The walrus build in this container accepts only ONE sync-wait per
    instruction; Tile emits more. Move extras onto same-engine NOPs placed
    immediately before the instruction (same-engine streams are in-order, so
    this is semantically identical)."""
    for bb in nc.main_func.blocks:
        insts = bb.instructions
        i = 0
        while i < len(insts):
            ins = insts[i]
            si = ins.sync_info
            waits = list(si.on_wait) if si is not None else []
            if len(waits) > 1:
                for w in waits[:-1]:
                    nop = mybir.InstNoOp(
                        name=nc.get_next_instruction_name(),
                        engine=ins.engine,
                        bass_nofuse=True,
                        sync_info=mybir.SyncInfo(on_wait=[w], on_update=[]),
                    )
                    nc.register_instruction(nop, overwrite=True)
                    insts.insert(i, nop)
                    i += 1
                ins.sync_info = mybir.SyncInfo(
                    on_wait=[waits[-1]], on_update=list(si.on_update)
                )
            i += 1


def _install_profile_hook():
    """Register the NTFF profile hook the agent image's antenv lacks, so
    run_bass_kernel_spmd(trace=True) can return HW exec times."""
    try:
        import antenv.axon_hooks  # noqa: F401
        return
    except ImportError:
        pass
    hook = None
    try:
        from trn_agent_boot.trn_boot import _ntff_profile_via_ctypes
        hook = _ntff_profile_via_ctypes("/opt/axon/libaxon_pjrt.so")
    except Exception:
        hook = None
    m = types.ModuleType("antenv.axon_hooks")
    m.get_axon_ntff_profile_hook = lambda: hook
    m.set_axon_ntff_profile_hook = lambda h: None
    sys.modules["antenv.axon_hooks"] = m


def hlo_sl(h):
    return slice(64 * (h % 2), 64 * (h % 2) + 64)


# ---------------------------------------------------------------- program
def build_program():
    nc = bass.Bass()

    # all inputs host-pre-tiled AND host-pre-converted to bf16
    xT = nc.declare_dram_parameter("xT", [128, KT * S], dt.bfloat16, isOutput=False)
    wqkvT = nc.declare_dram_parameter("wqkvT", [128, KT * MQKV], dt.bfloat16, isOutput=False)
    woT = nc.declare_dram_parameter("woT", [128, 2 * S], dt.bfloat16, isOutput=False)
    cosT = nc.declare_dram_parameter("cosT", [128, S], dt.bfloat16, isOutput=False)
    sinT = nc.declare_dram_parameter("sinT", [128, S], dt.bfloat16, isOutput=False)
    poutT = nc.declare_dram_parameter("poutT", [H, S], dt.bfloat16, isOutput=True)

    with tile.TileContext(nc) as tc, ExitStack() as stack:
        # ---------------- persistent pools / consts ----------------
        const_pool = stack.enter_context(tc.tile_pool(name="const", bufs=1))

        trig_pool = stack.enter_context(tc.tile_pool(name="trig", bufs=1))
        cos_rep = trig_pool.tile([128, S], dt.bfloat16, tag="cosr")
        sin_rep = trig_pool.tile([128, S], dt.bfloat16, tag="sinr")
        nc.gpsimd.dma_start(cos_rep[:], cosT[:])
        nc.gpsimd.dma_start(sin_rep[:], sinT[:])

        wo_pool = stack.enter_context(tc.tile_pool(name="wop", bufs=1))
        wo_b = [wo_pool.tile([128, S], dt.bfloat16, tag=f"wo{k}", name=f"wo{k}")
                for k in range(2)]
        for k in range(2):
            nc.gpsimd.dma_start(wo_b[k][:], woT[:, S * k:S * (k + 1)])

        # attention operand tiles
        att_pool = stack.enter_context(tc.tile_pool(name="att", bufs=1))
        qrope = [att_pool.tile([128, S], dt.bfloat16, tag=f"qrope{p}", name=f"qrope{p}")
                 for p in range(2)]
        kropeE = att_pool.tile([128, S], dt.bfloat16, tag="kropeE")
        kropeO = att_pool.tile([128, S], dt.bfloat16, tag="kropeO")
        nc.gpsimd.memset(kropeE[64:128, :], 0.0)
        nc.gpsimd.memset(kropeO[0:64, :], 0.0)
        vextA = att_pool.tile([128, S], dt.bfloat16, tag="vextA")
        vextB = att_pool.tile([128, S], dt.bfloat16, tag="vextB")
        nc.gpsimd.memset(vextA[:], 1.0)
        nc.gpsimd.memset(vextB[:], 1.0)
        vT_sb = att_pool.tile([128, S], dt.bfloat16, tag="vTsb")
        attnT = [att_pool.tile([128, S], dt.bfloat16, tag=f"attnT{p}", name=f"attnT{p}")
                 for p in range(2)]
        vA3 = vextA.rearrange("p (kb j) -> p kb j", kb=KB)

        # attention-phase SBUF scratch (persistent across phases)
        esb = stack.enter_context(tc.tile_pool(name="exp_sb", bufs=4))
        nsb = stack.enter_context(tc.tile_pool(name="norm_sb", bufs=2))
        osb = stack.enter_context(tc.tile_pool(name="out_sb", bufs=2))
        pout3 = poutT.rearrange("(mm p) j -> p mm j", p=128)

        # ---------------- x / wqkv loads (SBUF pool spans A0..A1) ----------
        xw_scope = ExitStack()
        proj_pool = xw_scope.enter_context(tc.tile_pool(name="proj", bufs=1))
        wqkv_big = proj_pool.tile([128, KT * MQKV], dt.bfloat16, tag="wqkvb")
        xt_pool = xw_scope.enter_context(tc.tile_pool(name="xtb", bufs=1))
        # x as [128, 1024] tiles: (k, half); half 0 loads first on both queues
        xt2 = [[xt_pool.tile([128, 1024], dt.bfloat16, tag=f"xt{k}h{hf}",
                             name=f"xt{k}h{hf}") for hf in range(2)]
               for k in range(KT)]
        rsc = xw_scope.enter_context(tc.tile_pool(name="rope_sc", bufs=2))

        def q_eng(i):
            return nc.sync if i % 2 == 0 else nc.scalar

        # wqkv chunk ch covers k in [2ch, 2ch+2); interleave with x so the
        # weights arrive just ahead of the x tiles that need them
        for ch in range(2):
            q_eng(ch).dma_start(
                wqkv_big[:, 2 * ch * MQKV:2 * (ch + 1) * MQKV],
                wqkvT[:, 2 * ch * MQKV:2 * (ch + 1) * MQKV])
        for k in range(KT):
            if k in (1, 3, 5):
                for ch in (k + 1, k + 2):
                    q_eng(ch).dma_start(
                        wqkv_big[:, 2 * ch * MQKV:2 * (ch + 1) * MQKV],
                        wqkvT[:, 2 * ch * MQKV:2 * (ch + 1) * MQKV])
            q_eng(k).dma_start(
                xt2[k][0][:], xT[:, k * S:k * S + 1024])

        def load_x_half1():
            # issued after the half-0 rope so these 16 descriptor issues
            # don't occupy the scalar engine ahead of the rope casts; the
            # sync queue has ~40us of slack before half 1 is consumed
            for k in range(KT):
                nc.sync.dma_start(
                    xt2[k][1][:], xT[:, k * S + 1024:k * S + 2048])

        def wqkv_sl(k, m):
            return wqkv_big[:, k * MQKV + 128 * m:k * MQKV + 128 * (m + 1)]

        # ---------------- phase A: qkv + RoPE for one sequence half --------
        def qkv_half(half, qpsum):
            # six [128,512] psum groups, one per (m, n2); n2=0 groups stop
            # halfway through the phase so the ACT casts (and the whole
            # RoPE chain) start while the n2=1 matmuls still stream
            pss = {}
            for m in (2, 0, 1):
                for n2 in range(2):
                    pss[(m, n2)] = qpsum.tile(
                        [128, 512], dt.float32, tag=f"qkvps{m}n{n2}",
                        name=f"qkvps{m}n{n2}")
            # k/v and the first q pair stream first (k-interleaved, paced to
            # DMA arrival); m=1 runs after, so its matmuls overlap the
            # DVE/ACT rope work on the m=2/m=0 outputs
            for n2 in range(2):
                for k in range(KT):
                    for m in (2, 0):
                        nc.tensor.matmul(
                            pss[(m, n2)][:],
                            wqkv_sl(k, m),
                            xt2[k][half][:, 512 * n2:512 * (n2 + 1)],
                            start=(k == 0), stop=(k == KT - 1),
                        )
            for n2 in range(2):
                for k in range(KT):
                    nc.tensor.matmul(
                        pss[(1, n2)][:],
                        wqkv_sl(k, 1),
                        xt2[k][half][:, 512 * n2:512 * (n2 + 1)],
                        start=(k == 0), stop=(k == KT - 1),
                    )
            # n2=0 chunks first: attention chunk q0/q1 depends on them
            for m, n2 in ((2, 0), (0, 0), (1, 0), (2, 1), (0, 1), (1, 1)):
                nrows = 128 if m < 2 else 64
                ps = pss[(m, n2)]
                n = 2 * half + n2
                sl = slice(512 * n, 512 * (n + 1))
                if m == 2:
                    nc.scalar.copy(vT_sb[64:128, sl], ps[64:128, :])
                qc = rsc.tile([128, 512], dt.float32, tag="qc", name="qc")
                nc.vector.tensor_tensor(out=qc[:nrows, :], in0=ps[:nrows, :],
                                        in1=cos_rep[:nrows, sl], op=ALU.mult)
                qraw = rsc.tile([128, 512], dt.bfloat16, tag="qraw", name="qraw")
                nc.scalar.copy(qraw[:nrows, :], ps[:nrows, :])
                # rotate_half is a signed partition swap (p <-> p^32); the
                # sign lives in the host sin table, so the rotation is just
                # partition-shift DMAs of the (lossless) bf16 qraw — no PE
                # matmul, no psum bank
                rotb = rsc.tile([128, 512], dt.bfloat16, tag="rotb", name="rotb")
                for blk in range(nrows // 64):
                    b0 = 64 * blk
                    nc.sync.dma_start(rotb[b0:b0 + 32, :],
                                      qraw[b0 + 32:b0 + 64, :])
                    nc.sync.dma_start(rotb[b0 + 32:b0 + 64, :],
                                      qraw[b0:b0 + 32, :])
                qs = rsc.tile([128, 512], dt.float32, tag="qs", name="qs")
                nc.vector.tensor_tensor(out=qs[:nrows, :], in0=rotb[:nrows, :],
                                        in1=sin_rep[:nrows, sl], op=ALU.mult)
                dst = qrope[m] if m < 2 else kropeE
                nc.vector.tensor_tensor(out=dst[:nrows, sl], in0=qc[:nrows, :],
                                        in1=qs[:nrows, :], op=ALU.add)
            # k duplicate + v transposes for this half
            hsl = slice(1024 * half, 1024 * (half + 1))
            nc.gpsimd.dma_start(kropeO[64:128, hsl], kropeE[0:64, hsl])
            nc.sync.dma_start_transpose(
                vA3[:, 8 * half:8 * (half + 1), 0:64], vT_sb[64:128, hsl])
            for kb in range(8 * half, 8 * half + 8):
                nc.gpsimd.dma_start(
                    vextB[:, 128 * kb + 64:128 * (kb + 1)],
                    vextA[:, 128 * kb:128 * kb + 64])

        # -------- phase B/D: attention + o_proj for one 512-col chunk ------
        def attn_chunk(q, spsum, vpsum, filler=None):
            """filler: list of closures (o_proj m-tile units for an earlier,
            already-normalized chunk); one is emitted after each kb2 pair so
            the in-order PE always has independent matmuls to run while this
            chunk's PV waits on ACT's exp."""
            filler = list(filler) if filler else []
            n_fill_total = len(filler)
            n_units_total = 4 * (4 * q + 4) // 2
            unit_idx = [0]
            fill_done = [0]
            qsl = slice(512 * q, 512 * (q + 1))
            nkb = 4 * q + 4          # kb blocks this chunk attends to
            pv_even = None
            for h in range(4):
                pair = h // 2
                par = h % 2      # 0: even head (pv rows 0-63), 1: odd
                krope = kropeE if par == 0 else kropeO
                vext = vextA if par == 0 else vextB
                pv = vpsum.tile([128, 512], dt.float32, tag="pv", name="pv")

                def emit_pv(kb2, ex, los):
                    for j in (0, 1):
                        kb = kb2 + j
                        lo = los[j]
                        nc.tensor.matmul(
                            pv[:, lo:512],
                            vext[:, 128 * kb:128 * (kb + 1)],
                            ex[:, 512 * j + lo:512 * (j + 1)],
                            start=(kb == 0), stop=(kb == nkb - 1),
                            skip_group_check=True)

                # software pipeline: scores/exp for pair i+1 are emitted
                # before the PV of pair i, so the in-order PE never waits
                # on ACT's exp of the pair it just scored
                pending = None
                for kb2 in range(0, nkb, 2):
                    sc = spsum.tile([128, 1024], dt.float32, tag="scps",
                                    name="scps")
                    ex = esb.tile([128, 1024], dt.bfloat16, tag="expp",
                                  name="expp")
                    los = []
                    for j in (0, 1):
                        kb = kb2 + j
                        lo = 128 * (kb - 4 * q) if kb >= 4 * q else 0
                        los.append(lo)
                        nc.tensor.matmul(
                            sc[:, 512 * j + lo:512 * (j + 1)],
                            krope[:, 128 * kb:128 * (kb + 1)],
                            qrope[pair][:, 512 * q + lo:512 * (q + 1)],
                            start=True, stop=True)
                    if los == [0, 0]:
                        nc.scalar.activation(ex[:], sc[:], AF.Exp,
                                             scale=float(SCALE))
                    else:
                        for j in (0, 1):
                            nc.scalar.activation(
                                ex[:, 512 * j + los[j]:512 * (j + 1)],
                                sc[:, 512 * j + los[j]:512 * (j + 1)],
                                AF.Exp, scale=float(SCALE))
                    for j in (0, 1):
                        kb = kb2 + j
                        if kb >= 4 * q:
                            # triangular causal mask on the [128,128]
                            # diagonal block: keep iff col >= partition
                            lo = los[j]
                            nc.gpsimd.affine_select(
                                out=ex[:, 512 * j + lo:512 * j + lo + 128],
                                in_=ex[:, 512 * j + lo:512 * j + lo + 128],
                                compare_op=ALU.is_ge, fill=0.0,
                                base=0, pattern=[[1, 128]],
                                channel_multiplier=-1)
                    # Bresenham-spread the filler units across the chunk,
                    # between this pair's scores and the previous pair's PV:
                    # the PE chews the filler while ACT exps the new scores
                    unit_idx[0] += 1
                    want = unit_idx[0] * n_fill_total // n_units_total
                    while fill_done[0] < want and filler:
                        filler.pop(0)()
                        fill_done[0] += 1
                    if pending is not None:
                        emit_pv(*pending)
                    pending = (kb2, ex, los)
                if pending is not None:
                    emit_pv(*pending)
                if par == 0:
                    pv_even = pv
                    continue
                # paired normalize for heads (h-1, h): one Ln + one Exp on
                # a full [128,512] tile instead of two per-head passes.
                # pv_even: pv rows 0-63, sums 64-127; pv (odd): flipped.
                pvmix = nsb.tile([128, 512], dt.float32, tag="pvmix", name="pvmix")
                nc.vector.tensor_copy(pvmix[0:64, :], pv_even[0:64, :])
                nc.vector.tensor_copy(pvmix[64:128, :], pv[64:128, :])
                summix = nsb.tile([128, 512], dt.float32, tag="summix", name="summix")
                nc.vector.tensor_copy(summix[0:64, :], pv[0:64, :])
                nc.vector.tensor_copy(summix[64:128, :], pv_even[64:128, :])
                lns = nsb.tile([128, 512], dt.float32, tag="lns", name="lns")
                nc.scalar.activation(lns[:], summix[:], AF.Ln)
                # partition-swap the halves so each rcp row aligns with its pv
                lnd = nsb.tile([128, 512], dt.float32, tag="lnd", name="lnd")
                # sync queue, not gpsimd: keep gpsimd free for the affines
                nc.sync.dma_start(lnd[0:64, :], lns[64:128, :])
                nc.sync.dma_start(lnd[64:128, :], lns[0:64, :])
                rcp = nsb.tile([128, 512], dt.float32, tag="rcp", name="rcp")
                nc.scalar.activation(rcp[:], lnd[:], AF.Exp, scale=-1.0)
                nc.vector.tensor_tensor(
                    out=attnT[pair][:, qsl],
                    in0=pvmix[:], in1=rcp[:], op=ALU.mult)
            while filler:
                filler.pop(0)()

        def oproj_units(q, opsum, copy_eng="mix"):
            """16 closures, one per o_proj m-tile of chunk q (2 matmuls +
            psum->bf16 copy; every 8th also fires the staged out-DMA).
            copy_eng='dve' keeps all copies off ACT — use for filler units
            that run inside an exp-saturated attention chunk."""
            qsl = slice(512 * q, 512 * (q + 1))
            state = {}

            def make_unit(m):
                mh, mm = m // 8, m % 8

                def unit():
                    if mm == 0:
                        state[mh] = osb.tile([128, 8 * 512], dt.bfloat16,
                                             tag="ob", name="ob")
                    ob = state[mh]
                    ps = opsum.tile([128, 512], dt.float32, tag="ops", name="ops")
                    for kd in range(2):
                        nc.tensor.matmul(
                            ps[:],
                            wo_b[kd][:, 128 * m:128 * (m + 1)],
                            attnT[kd][:, qsl],
                            start=(kd == 0), stop=(kd == 1))
                    if copy_eng == "dve" or mm % 2 == 0:
                        nc.vector.tensor_copy(ob[:, 512 * mm:512 * (mm + 1)], ps[:])
                    else:
                        nc.scalar.copy(ob[:, 512 * mm:512 * (mm + 1)], ps[:])
                    if mm == 7:
                        ob3 = ob.rearrange("p (mm j) -> p mm j", mm=8)
                        if q == NS - 1 and mh == 1:
                            # last transfer: split across both queues
                            nc.sync.dma_start(pout3[:, 8:12, qsl], ob3[:, 0:4, :])
                            nc.scalar.dma_start(pout3[:, 12:16, qsl], ob3[:, 4:8, :])
                        else:
                            eng = nc.sync if (q + mh) % 2 == 0 else nc.scalar
                            eng.dma_start(pout3[:, 8 * mh:8 * (mh + 1), qsl],
                                          ob3[:, :, :])
                return unit

            return [make_unit(m) for m in range(KT)]

        def oproj_chunk(q, opsum):
            for u in oproj_units(q, opsum):
                u()

        # ---------------- the pipeline ----------------
        with tc.tile_pool(name="qkv_psum0", bufs=1, space="PSUM") as qpsum0:
            qkv_half(0, qpsum0)
            load_x_half1()
        with tc.tile_pool(name="sc_psum0", bufs=2, space="PSUM") as spsum0, \
             tc.tile_pool(name="pv_psum0", bufs=2, space="PSUM") as vpsum0, \
             tc.tile_pool(name="op_psum0", bufs=2, space="PSUM") as opsum0:
            attn_chunk(0, spsum0, vpsum0)
            attn_chunk(1, spsum0, vpsum0, filler=oproj_units(0, opsum0))
            # o_proj q1 is deferred into phase D as attn q2's filler, so
            # phase B's psum drains fast and A1 isn't blocked on bank WARs
        with tc.tile_pool(name="qkv_psum1", bufs=1, space="PSUM") as qpsum1:
            qkv_half(1, qpsum1)
        xw_scope.close()
        with tc.tile_pool(name="sc_psum1", bufs=2, space="PSUM") as spsum1, \
             tc.tile_pool(name="pv_psum1", bufs=2, space="PSUM") as vpsum1, \
             tc.tile_pool(name="op_psum1", bufs=2, space="PSUM") as opsum1:
            attn_chunk(2, spsum1, vpsum1, filler=oproj_units(1, opsum1))
            attn_chunk(3, spsum1, vpsum1, filler=oproj_units(2, opsum1))
            oproj_chunk(3, opsum1)

    _split_multi_waits(nc)
    return nc


_PROGRAM = None


def _get_program():
    global _PROGRAM
    if _PROGRAM is None:
        _PROGRAM = build_program()
    return _PROGRAM


# ---------------------------------------------------------------- host side
def make_inputs(hidden_states, position_ids, wq, wk, wv, wo):
    """Shard + marshal full inputs into per-core DRAM parameter maps.

    All dtype conversion (fp32 -> bf16) and the RoPE trig tables are done
    here on the host; the device kernel only sees bf16 operands."""
    x = np.asarray(hidden_states, dtype=np.float32).reshape(S, H)
    # pre-tiled [128, KT*S]: row p, col k*S+j  =  xT[k*128+p, j] = x[j, k*128+p]
    xT = np.ascontiguousarray(
        x.T.reshape(KT, 128, S).transpose(1, 0, 2).reshape(128, KT * S)
    ).astype(bf16)

    # RoPE trig tables [128, S]: partition p covers q/k dim (p % 64) of a
    # head; inv_freq index is (p % 64) % 32 == p % 32
    pos = np.asarray(position_ids).reshape(S).astype(np.float64)
    inv_freq = 1.0 / (ROPE_BASE ** (np.arange(0, D, 2, dtype=np.float64) / D))
    ang = pos[None, :] * inv_freq[np.arange(128) % 32][:, None]  # [128, S]
    cosT = np.cos(ang).astype(bf16)
    # rotate_half sign folded in: rows p%64<32 multiply the swapped value
    # by -sin, rows p%64>=32 by +sin
    sign = np.where(np.arange(128) % 64 < 32, -1.0, 1.0)[:, None]
    sinT = (np.sin(ang) * sign).astype(bf16)

    wq = np.asarray(wq, dtype=np.float32)
    wk = np.asarray(wk, dtype=np.float32)
    wv = np.asarray(wv, dtype=np.float32)
    wo = np.asarray(wo, dtype=np.float32)

    in_maps = []
    for c in range(N_CORES):
        wq_c = wq[DQ * c:DQ * (c + 1)]           # [256, H]
        wk_c = wk[D * c:D * (c + 1)]             # [64, H]
        wv_c = wv[D * c:D * (c + 1)]             # [64, H]
        wqkvT_c = np.concatenate([wq_c, wk_c, wv_c], axis=0).T   # [H, 384]
        wqkvT_c = np.ascontiguousarray(
            wqkvT_c.reshape(KT, 128, MQKV).transpose(1, 0, 2)
            .reshape(128, KT * MQKV)).astype(bf16)
        woT_c = wo[:, DQ * c:DQ * (c + 1)].T                 # [256, H]
        woT_c = np.ascontiguousarray(
            woT_c.reshape(2, 128, H).transpose(1, 0, 2).reshape(128, 2 * H)
        ).astype(bf16)
        in_maps.append({
            "xT": xT,
            "wqkvT": wqkvT_c,
            "woT": woT_c,
            "cosT": cosT,
            "sinT": sinT,
        })
    return in_maps


def kernel(hidden_states, position_ids, wq, wk, wv, wo):
    _install_profile_hook()
    nc = _get_program()
    in_maps = make_inputs(hidden_states, position_ids, wq, wk, wv, wo)
    res = run_bass_kernel_spmd(nc, in_maps, list(range(N_CORES)))
    acc = np.zeros((H, S), dtype=np.float32)
    for c in range(N_CORES):
        acc += res.results[c]["poutT"].astype(np.float32)
    return np.ascontiguousarray(acc.T)[None, :, :]


if __name__ == "__main__":
    rng = np.random.default_rng(0)
    hs = rng.standard_normal((1, S, H), dtype=np.float32)
    pid = np.broadcast_to(np.arange(S, dtype=np.int64)[None, :], (1, S))
    std = 1.0 / np.sqrt(H)
    w_q = (rng.standard_normal((NH * D, H), dtype=np.float32) * std)
    w_k = (rng.standard_normal((NKV * D, H), dtype=np.float32) * std)
    w_v = (rng.standard_normal((NKV * D, H), dtype=np.float32) * std)
    w_o = (rng.standard_normal((H, NH * D), dtype=np.float32) * std)
    out = kernel(hs, pid, w_q, w_k, w_v, w_o)
    print("out", out.shape, out.dtype, float(np.abs(out).mean()))
